# revision 2
# baseline (speedup 1.0000x reference)
"""Trainium2 Bass kernel for nn_EnhancedDetector (GNN message passing), 8 cores.

Dev version: imports gnn_lib. The final submission inlines everything.
"""
import os, sys
sys.path.insert(0, '/opt/trn_rl_repo')
sys.path.insert(0, os.path.dirname(os.path.abspath(__file__)))
import numpy as np

import gnn_lib
from gnn_lib import Cfg, host_prep, build_program, make_in_maps

LAST_EXEC_NS = None
_CACHE = {}


def kernel(**inputs) -> np.ndarray:
    global LAST_EXEC_NS
    from concourse.bass_utils import run_bass_kernel_spmd

    cfg = Cfg(N=50000, E=800000, G=64)
    x = np.asarray(inputs["x"], np.float32)
    edge_index = np.asarray(inputs["edge_index"])
    batch = np.asarray(inputs["batch"])

    meta, percore, shared = host_prep(x, edge_index, batch, cfg)
    key = (meta["TOT"], meta["ICOLS"])
    if key not in _CACHE:
        _CACHE[key] = build_program(cfg, meta, debug_taps=False, act="gelu")
    nc = _CACHE[key]
    in_maps = make_in_maps(inputs, cfg, meta, percore, shared)
    in_maps = [{k: np.ascontiguousarray(v) for k, v in m.items()} for m in in_maps]

    if os.environ.get("GNN_BENCH", "0") == "1":
        results, times = gnn_lib.bench_exec(nc, in_maps, n_cores=cfg.ncores,
                                            iters=8)
        LAST_EXEC_NS = int(min(times) * 1e9)
        print("bench times (s):", [f"{t:.4f}" for t in times])
        return np.asarray(results[0]["z"], np.float32)
    r = run_bass_kernel_spmd(nc, in_maps, core_ids=list(range(cfg.ncores)))
    LAST_EXEC_NS = r.exec_time_ns
    return np.asarray(r.results[0]["z"], np.float32)


# revision 3
# speedup vs baseline: 26.3588x; 26.3588x over previous
"""Trainium2 Bass kernel for nn_EnhancedDetector (GNN message passing), 8 cores.

Dev version: imports gnn_lib. The final submission inlines everything.
"""
import os, sys
sys.path.insert(0, '/opt/trn_rl_repo')
sys.path.insert(0, os.path.dirname(os.path.abspath(__file__)))
import numpy as np

import gnn_lib
from gnn_lib import Cfg, host_prep, build_program, make_in_maps

LAST_EXEC_NS = None
_CACHE = {}


def kernel(**inputs) -> np.ndarray:
    global LAST_EXEC_NS
    from concourse.bass_utils import run_bass_kernel_spmd

    cfg = Cfg(N=50000, E=800000, G=64)
    x = np.asarray(inputs["x"], np.float32)
    edge_index = np.asarray(inputs["edge_index"])
    batch = np.asarray(inputs["batch"])

    meta, percore, shared = host_prep(x, edge_index, batch, cfg)
    key = (meta["TOT"], meta["ICOLS"])
    if key not in _CACHE:
        _CACHE[key] = build_program(cfg, meta, debug_taps=False, act="gelu")
    nc = _CACHE[key]
    in_maps = make_in_maps(inputs, cfg, meta, percore, shared)
    in_maps = [{k: np.ascontiguousarray(v) for k, v in m.items()} for m in in_maps]

    if os.environ.get("GNN_TRACE", "0") == "1":
        import tempfile
        gnn_lib.install_ntff_hook()
        td = tempfile.mkdtemp(prefix="gnntrace_")
        print("trace dir:", td)
        r = run_bass_kernel_spmd(nc, in_maps, core_ids=list(range(cfg.ncores)),
                                 trace=True, tmpdir=td)
        LAST_EXEC_NS = r.exec_time_ns
        return np.asarray(r.results[0]["z"], np.float32)
    if os.environ.get("GNN_BENCH", "0") == "1":
        results, times = gnn_lib.bench_exec(nc, in_maps, n_cores=cfg.ncores,
                                            iters=8)
        LAST_EXEC_NS = int(min(times) * 1e9)
        print("bench times (s):", [f"{t:.4f}" for t in times])
        return np.asarray(results[0]["z"], np.float32)
    r = run_bass_kernel_spmd(nc, in_maps, core_ids=list(range(cfg.ncores)))
    LAST_EXEC_NS = r.exec_time_ns
    return np.asarray(r.results[0]["z"], np.float32)
